# revision 1
# baseline (speedup 1.0000x reference)
"""Trainium2 Bass kernel for nn_CLS_1889785610440.

Pipeline (per reference.py):
  3 scalar Elman RNNs over T in {4,8,16} for N=B*M*E lanes -> last hidden
  -> 1x3 conv over scales -> scalar RNN over M=64 -> BatchNorm1d (batch
  stats) -> ReLU -> Linear(E,C) -> softmax.

Sharding: data-parallel over the batch dim B=128 -> 16 samples per core
(contiguous N/8 lane chunks of a0/a1/a2). Only the BatchNorm statistics
cross cores (one 2KB AllReduce).

Device mapping:
  - stage-1 recurrence step:  psum = diag(wih_s) @ x_t + diag(whh_s) @ h
    on TensorE (two accumulating matmuls per step, 128x512 tiles), then
    h = tanh(psum + b) on ScalarE.  VectorE stays free.
  - conv: 3 accumulating diag matmuls; ScalarE copy folds rnn2's input
    scale/bias so the result is directly rnn2's per-step input u2.
  - rnn2: PE-transpose 128x128 blocks so lanes=(b_loc,e) sit on
    partitions (128 x 32 tile); per step one fused DVE
    scalar_tensor_tensor (h*whh2 + u2_m) + ScalarE tanh.  When
    |whh2| < 1 the recurrence is truncated to K steps with
    |whh2|^K < 1e-6 (only the last hidden state is needed).
  - BN: per-core sum/sumsq -> AllReduce(128x4) -> mean/var; inv_std via
    exp(-0.5*ln(var+eps)) (stays in the ln/exp ACT table set);
    normalize+relu on DVE; FC via two matmuls; softmax on-device.
"""

import numpy as np

import concourse.bacc as bacc
import concourse.tile as tile
import concourse.mybir as mybir
from concourse.bass_utils import run_bass_kernel_spmd

# Problem constants (hardcoded per spec).
B = 128
E = 256
M = 64
S = 3
C = 5
SCALES = [4, 8, 16]
EPS = 1e-5

N_CORES = 8
N = B * M * E              # 2097152 lanes
N8 = N // N_CORES          # 262144 lanes per core
F = 512                    # free dim of a stage-1 tile
NCHUNK = N8 // (128 * F)   # 4 chunks of (128, 512) lanes per core
BLOC = B // N_CORES        # 16 samples per core
L2 = BLOC * 2              # 32 rnn2 lanes per partition

FP32 = mybir.dt.float32
AF = mybir.ActivationFunctionType
ALU = mybir.AluOpType


def _build(params, repeat=1, no_collective=False, n_devices=N_CORES,
           k_override=None, bench_internal=False):
    """Build the Bass program. `params` holds host-side python floats and
    small numpy arrays derived from the model parameters.  `repeat` re-emits
    the whole compute body K times (benchmarking only — differential timing
    against repeat=1 cancels the host<->device transfer baseline)."""
    nc = bacc.Bacc("TRN2", target_bir_lowering=False, debug=False,
                   enable_asserts=True, num_devices=n_devices)

    a_kind = "Internal" if bench_internal else "ExternalInput"
    a_dram = [
        nc.dram_tensor(f"a{i}", [N8 * T], mybir.dt.float32r, kind=a_kind)
        for i, T in enumerate(SCALES)
    ]
    out_dram = nc.dram_tensor("out", [BLOC, C], FP32, kind="ExternalOutput")

    # Inline constants (baked into the NEFF, replicated on every core).
    # diag blocks: [wih0..2 | whh0..2 | cw0..2 | identity] as 128x128 fp32.
    eye = np.eye(128, dtype=np.float32)
    diag_blocks = (
        [eye * params["wih"][s] for s in range(S)]
        + [eye * params["whh"][s] for s in range(S)]
        + [eye * params["cw"][s] for s in range(S)]
        + [eye]
    )
    diag_np = np.concatenate(diag_blocks, axis=1)  # (128, 128*10)
    diag_c = nc.inline_tensor(diag_np, name="diagc")

    # FC weights packed for contraction over e_lo: W[e_lo, eh*C + c] =
    # fnn_w[c, eh*128 + e_lo]
    fw = params["fnn_w"]  # (C, E)
    wpack_np = np.concatenate(
        [fw[:, :128].T.astype(np.float32), fw[:, 128:].T.astype(np.float32)],
        axis=1)  # (128, 2C)
    wpack_c = nc.inline_tensor(wpack_np, name="wpack")

    # gamma/beta arranged (e_lo, e_hi):  [gamma | beta] -> (128, 4)
    g = params["gamma"].reshape(2, 128).T.astype(np.float32)
    bta = params["beta"].reshape(2, 128).T.astype(np.float32)
    gb_c = nc.inline_tensor(np.concatenate([g, bta], axis=1), name="gb")

    fnnb_c = nc.inline_tensor(
        params["fnn_b"].reshape(C, 1).astype(np.float32), name="fnnb")

    wih2 = params["wih2"]
    whh2 = params["whh2"]
    bias2 = wih2 * params["cb"] + params["bb2"]

    # activation biases as per-partition columns: [bb0, bb1, bb2, bias2, EPS]
    bias_np = np.tile(
        np.array([params["bb"][0], params["bb"][1], params["bb"][2],
                  bias2, EPS], np.float32)[None, :], (128, 1))
    bias_c = nc.inline_tensor(bias_np, name="biasc")

    # rnn2 truncation: error of last hidden <= |whh2|^K
    aw = abs(whh2)
    if aw < 1e-12:
        K = 1
    elif aw >= 1.0:
        K = M
    else:
        K = min(M, max(1, int(np.ceil(np.log(1e-6) / np.log(aw)))))
    if k_override is not None:
        K = k_override

    from contextlib import ExitStack
    with tile.TileContext(nc) as tc, ExitStack() as ctx:
        singles = ctx.enter_context(tc.tile_pool(name="singles", bufs=1))
        xp = [ctx.enter_context(tc.tile_pool(name=f"x{s}", bufs=2))
              for s in range(S)]
        hp = ctx.enter_context(tc.tile_pool(name="h", bufs=6))
        hfp = ctx.enter_context(tc.tile_pool(name="hf", bufs=2))
        cvp = ctx.enter_context(tc.tile_pool(name="cv", bufs=2))
        r2p = ctx.enter_context(tc.tile_pool(name="r2", bufs=1))
        smp = ctx.enter_context(tc.tile_pool(name="sm", bufs=2))
        ps1 = ctx.enter_context(tc.tile_pool(name="ps1", bufs=5, space="PSUM"))
        psc = ctx.enter_context(tc.tile_pool(name="psc", bufs=1, space="PSUM"))
        pst = ctx.enter_context(tc.tile_pool(name="pst", bufs=1, space="PSUM"))
        dram = ctx.enter_context(tc.tile_pool(name="dram", bufs=1, space="DRAM"))

        diag_sb = singles.tile([128, 128 * 10], FP32)
        nc.sync.dma_start(out=diag_sb[:], in_=diag_c[:])
        wpack_sb = singles.tile([128, 2 * C], FP32)
        nc.sync.dma_start(out=wpack_sb[:], in_=wpack_c[:])
        gb_sb = singles.tile([128, 4], FP32)
        nc.sync.dma_start(out=gb_sb[:], in_=gb_c[:])
        fnnb_sb = singles.tile([C, 1], FP32)
        nc.sync.dma_start(out=fnnb_sb[:], in_=fnnb_c[:])
        bias_sb = singles.tile([128, 5], FP32)
        nc.sync.dma_start(out=bias_sb[:], in_=bias_c[:])

        # float32r: fp32 storage, 1 cycle/row on the PE at free>=256
        # (plain float32 costs 4 cycles/row).  Diagonal matmuls only round
        # one product per output so the precision impact is minimal.
        # walrus requires f32r matmul operands to be produced as f32r, so
        # x/h tiles are declared f32r and the diag weights are cast once.
        R32 = mybir.dt.float32r
        diag_r = singles.tile([128, 128 * 9], R32)
        nc.vector.tensor_copy(diag_r[:], diag_sb[:, 0:128 * 9])

        def dwih(s):
            return diag_r[:, s * 128:(s + 1) * 128]

        def dwhh(s):
            return diag_r[:, (S + s) * 128:(S + s + 1) * 128]

        def dcw(s):
            return diag_r[:, (2 * S + s) * 128:(2 * S + s + 1) * 128]

        ident = diag_sb[:, 3 * S * 128:(3 * S + 1) * 128]

        a_view = [
            a_dram[s].ap().rearrange("(c p ft) -> c p ft", c=NCHUNK, p=128)
            for s in range(S)
        ]

        for _rep in range(repeat):
            # rnn2 input u2, layout [e_lo, m, l] with l = b_loc*2 + e_hi
            rnn2buf = r2p.tile([128, M, L2], FP32, tag="rnn2buf",
                               name="rnn2buf")
            # rnn2 is split into two lane groups (chunks 0-1 -> columns
            # 0:16, chunks 2-3 -> 16:32) so the first half overlaps the
            # second half of stage 1.
            feat = smp.tile([128, L2], FP32, tag="feat", name="feat")

            def rnn2_group(g):
                lo, hi = 16 * g, 16 * g + 16
                h2 = None
                for m in range(M - K, M):
                    last = m == M - 1
                    dst = (feat[:, lo:hi] if last else
                           smp.tile([128, 16], FP32, tag=f"h2_{g}",
                                    name=f"h2_{g}")[:])
                    if h2 is None:
                        nc.scalar.activation(dst, rnn2buf[:, m, lo:hi],
                                             AF.Tanh)
                    else:
                        st = smp.tile([128, 16], FP32, tag=f"st_{g}",
                                      name=f"st_{g}")
                        nc.vector.scalar_tensor_tensor(
                            st[:], h2, whh2, rnn2buf[:, m, lo:hi],
                            op0=ALU.mult, op1=ALU.add)
                        nc.scalar.activation(dst, st[:], AF.Tanh)
                    h2 = dst

            for c in range(NCHUNK):
                xt = []
                for s, T in enumerate(SCALES):
                    x = xp[s].tile([128, F, T], R32, tag=f"x{s}",
                                   name=f"x{s}")
                    nc.sync.dma_start(
                        out=x[:].rearrange("p f t -> p (f t)"),
                        in_=a_view[s][c])
                    xt.append(x)

                # interleave the three scales' recurrences step by step
                h_cur = [None] * S
                hfin = []
                for s in range(S):
                    hfin.append(hfp.tile([128, F], R32, tag=f"hf{s}",
                                         name=f"hf{s}"))
                for t in range(max(SCALES)):
                    for s, T in enumerate(SCALES):
                        if t >= T:
                            continue
                        ps = ps1.tile([128, F], FP32, tag="ps", name="ps")
                        if t == 0:
                            nc.tensor.matmul(ps[:], dwih(s), xt[s][:, :, t],
                                             start=True, stop=True)
                        else:
                            nc.tensor.matmul(ps[:], dwih(s), xt[s][:, :, t],
                                             start=True, stop=False)
                            nc.tensor.matmul(ps[:], dwhh(s), h_cur[s][:],
                                             start=False, stop=True)
                        hn = hfin[s] if t == T - 1 else hp.tile(
                            [128, F], R32, tag=f"h{s}", name=f"h{s}")
                        nc.scalar.activation(hn[:], ps[:], AF.Tanh,
                                             bias=bias_sb[:, s:s + 1])
                        h_cur[s] = hn

                # conv over scales + fold rnn2 input affine:
                #   u2 = wih2*(sum_s cw_s*h_s + cb) + bih2 + bhh2
                pc = psc.tile([128, F], FP32, tag="pc", name="pc")
                nc.tensor.matmul(pc[:], dcw(0), hfin[0][:],
                                 start=True, stop=False)
                nc.tensor.matmul(pc[:], dcw(1), hfin[1][:],
                                 start=False, stop=False)
                nc.tensor.matmul(pc[:], dcw(2), hfin[2][:],
                                 start=False, stop=True)
                cv = cvp.tile([128, F], FP32, tag="cv", name="cv")
                nc.vector.tensor_scalar(cv[:], pc[:], wih2, bias2,
                                        op0=ALU.mult, op1=ALU.add)

                # transpose each 128x128 block; scatter into rnn2buf
                for j in range(4):
                    m_lo, e_hi = j // 2, j % 2
                    pt = pst.tile([128, 128], FP32, tag="pt", name="pt")
                    nc.tensor.transpose(pt[:], cv[:, j * 128:(j + 1) * 128],
                                        ident)
                    src = pt[:].rearrange("p (b v) -> p v b", b=4)
                    dst = rnn2buf[:, m_lo::2, 8 * c + e_hi:8 * c + 8:2]
                    nc.vector.tensor_copy(dst, src)
                if c == 1:
                    rnn2_group(0)
                elif c == 3:
                    rnn2_group(1)

            # ---- BatchNorm stats (partial) ----
            featsq = smp.tile([128, L2], FP32, tag="fsq", name="fsq")
            nc.vector.tensor_tensor(featsq[:], feat[:], feat[:], ALU.mult)
            stats = smp.tile([128, 4], FP32, tag="stats", name="stats")
            fv = feat[:].rearrange("p (b eh) -> p eh b", b=BLOC)
            fsv = featsq[:].rearrange("p (b eh) -> p eh b", b=BLOC)
            nc.vector.tensor_reduce(stats[:, 0:2], fv,
                                    axis=mybir.AxisListType.X, op=ALU.add)
            nc.vector.tensor_reduce(stats[:, 2:4], fsv,
                                    axis=mybir.AxisListType.X, op=ALU.add)

        bin_ = dram.tile([128, 4], FP32, tag="bin")
        bout = dram.tile([128, 4], FP32, tag="bout")
        nc.gpsimd.dma_start(bin_[:], stats[:])
        if no_collective:
            nc.gpsimd.dma_start(bout[:], bin_[:])
        else:
            nc.gpsimd.collective_compute(
                "AllReduce", ALU.add,
                replica_groups=[list(range(N_CORES))],
                ins=[bin_.opt()], outs=[bout.opt()])
        stg = smp.tile([128, 4], FP32, tag="stg")
        nc.gpsimd.dma_start(stg[:], bout[:])

        # mean/var/scale/shift (all (128,2): per (e_lo, e_hi))
        mean = smp.tile([128, 2], FP32, tag="mean")
        nc.vector.tensor_scalar(mean[:], stg[:, 0:2], 1.0 / B, None, ALU.mult)
        ex2 = smp.tile([128, 2], FP32, tag="ex2")
        nc.vector.tensor_scalar(ex2[:], stg[:, 2:4], 1.0 / B, None, ALU.mult)
        var = smp.tile([128, 2], FP32, tag="var")
        nc.vector.tensor_tensor(var[:], mean[:], mean[:], ALU.mult)
        nc.vector.tensor_tensor(var[:], ex2[:], var[:], ALU.subtract)
        lnv = smp.tile([128, 2], FP32, tag="lnv")
        nc.scalar.activation(lnv[:], var[:], AF.Ln, bias=bias_sb[:, 4:5])
        istd = smp.tile([128, 2], FP32, tag="istd")
        nc.scalar.activation(istd[:], lnv[:], AF.Exp, scale=-0.5)
        scl = smp.tile([128, 2], FP32, tag="scl")
        nc.vector.tensor_tensor(scl[:], istd[:], gb_sb[:, 0:2], ALU.mult)
        shf = smp.tile([128, 2], FP32, tag="shf")
        nc.vector.tensor_tensor(shf[:], mean[:], scl[:], ALU.mult)
        nc.vector.tensor_tensor(shf[:], gb_sb[:, 2:4], shf[:], ALU.subtract)

        # normalize + relu
        r = smp.tile([128, L2], FP32, tag="r")
        f3 = feat[:].rearrange("p (b eh) -> p b eh", b=BLOC)
        r3 = r[:].rearrange("p (b eh) -> p b eh", b=BLOC)
        for eh in range(2):
            nc.vector.tensor_scalar(
                r3[:, :, eh], f3[:, :, eh],
                scl[:, eh:eh + 1], shf[:, eh:eh + 1],
                op0=ALU.mult, op1=ALU.add)
        nc.vector.tensor_scalar_max(r[:], r[:], 0.0)

        # FC: logits^T (C, BLOC) = sum_eh Wpack_eh.T @ r[:, :, eh]
        tailps = pst.tile([128, 512], FP32, tag="tailps")
        pl = tailps[0:C, 0:BLOC]
        nc.tensor.matmul(pl, wpack_sb[:, 0:C], r3[:, :, 0],
                         start=True, stop=False)
        nc.tensor.matmul(pl, wpack_sb[:, C:2 * C], r3[:, :, 1],
                         start=False, stop=True)
        lt = smp.tile([C, BLOC], FP32, tag="lt")
        nc.vector.tensor_scalar(lt[:], pl, fnnb_sb[:, 0:1], None, ALU.add)

        # transpose to (BLOC, C) and softmax along free dim
        pt2 = tailps[0:BLOC, 128:128 + C]
        nc.tensor.transpose(pt2, lt[:], ident[0:C, 0:C])
        nmax = smp.tile([BLOC, 1], FP32, tag="nmax")
        nc.vector.tensor_reduce(nmax[:], pt2, axis=mybir.AxisListType.X,
                                op=ALU.max, negate=True)
        esb = smp.tile([BLOC, C], FP32, tag="esb")
        nc.scalar.activation(esb[:], pt2, AF.Exp, bias=nmax[:, 0:1])
        ssum = smp.tile([BLOC, 1], FP32, tag="ssum")
        nc.vector.tensor_reduce(ssum[:], esb[:], axis=mybir.AxisListType.X,
                                op=ALU.add)
        rin = smp.tile([BLOC, 1], FP32, tag="rin")
        nc.vector.reciprocal(rin[:], ssum[:])
        osb = smp.tile([BLOC, C], FP32, tag="osb")
        nc.vector.tensor_scalar(osb[:], esb[:], rin[:, 0:1], None, ALU.mult)
        nc.sync.dma_start(out=out_dram[:], in_=osb[:])

    nc.compile()
    return nc


def kernel(a0, a1, a2, rnn1_wih, rnn1_whh, rnn1_bih, rnn1_bhh,
           conv_w, conv_b, rnn2_wih, rnn2_whh, rnn2_bih, rnn2_bhh,
           norm_gamma, norm_beta, fnn_w, fnn_b, _bench=None):
    params = {
        "wih": [float(rnn1_wih[s]) for s in range(S)],
        "whh": [float(rnn1_whh[s]) for s in range(S)],
        "bb": [float(rnn1_bih[s]) + float(rnn1_bhh[s]) for s in range(S)],
        "cw": [float(conv_w[s]) for s in range(S)],
        "cb": float(conv_b[0]),
        "wih2": float(rnn2_wih[0]),
        "whh2": float(rnn2_whh[0]),
        "bb2": float(rnn2_bih[0]) + float(rnn2_bhh[0]),
        "gamma": np.asarray(norm_gamma, np.float32),
        "beta": np.asarray(norm_beta, np.float32),
        "fnn_w": np.asarray(fnn_w, np.float32),
        "fnn_b": np.asarray(fnn_b, np.float32),
    }
    nc = _build(params)

    flat = [np.ascontiguousarray(np.asarray(a, np.float32)).reshape(-1)
            for a in (a0, a1, a2)]
    in_maps = []
    for k in range(N_CORES):
        m = {}
        for i, T in enumerate(SCALES):
            sz = N8 * T
            m[f"a{i}"] = flat[i][k * sz:(k + 1) * sz]
        in_maps.append(m)

    kw = dict(_bench) if _bench else {}
    res = run_bass_kernel_spmd(nc, in_maps, core_ids=list(range(N_CORES)),
                               **kw)
    out = np.concatenate([res.results[k]["out"] for k in range(N_CORES)],
                         axis=0)
    if _bench is not None:
        kernel.last_result = res
    return out



# revision 2
# speedup vs baseline: 2.3448x; 2.3448x over previous
"""Trainium2 Bass kernel for nn_CLS_1889785610440.

Pipeline (per reference.py):
  3 scalar Elman RNNs over T in {4,8,16} for N=B*M*E lanes -> last hidden
  -> 1x3 conv over scales -> scalar RNN over M=64 -> BatchNorm1d (batch
  stats) -> ReLU -> Linear(E,C) -> softmax.

Key optimizations vs the v1 baseline:
  * Truncation-aware LOADING: the rnn2 recurrence over m contracts by
    |whh2| (=0.61) per step, so only the last Km ~ 12 of 64 m-positions
    influence the output above ~3e-3; stage-1 recurrences likewise only
    need their last Kt_s steps (|whh_s|^Kt_s <= 1e-3).  Lanes with
    m < M-Km and time-steps t < T-Kt_s are never uploaded or computed.
    (measured end-to-end error 2.2e-3 vs the 2e-2 gate)
  * bf16 upload: inputs are quantized host-side to bf16 (recurrence
    arithmetic stays fp32 on-chip), halving DMA bytes.
  * Host-side gather picks the lane order so stage-1 partitions are
    e_lo directly: free dim = (m, b_loc, e_hi).  The conv output IS the
    rnn2 input buffer - no PE transposes / scatters at all.
  * Stage-1 step on DVE+ACT: st = h*(whh/wih) + x_t (one
    scalar_tensor_tensor), h = tanh(wih*st + b) (one activation with
    folded scale+bias).  No matmuls, PSUM untouched until the FC.
  * Conv + rnn2 input affine folded into 2 DVE ops (pivot scale) and
    the rnn2 activation scale/bias.
  * BN normalize+relu fused into one ACT op per e_hi via per-partition
    scale/bias (relu(scl*f + shf)).
  * Single ACT table switch (Tanh -> Ln/Exp set) for the whole kernel.

Sharding: data-parallel over B=128 -> 16 samples/core.  Only the
BatchNorm statistics cross cores (one 2KB AllReduce).
"""

import math

import numpy as np
import ml_dtypes

import concourse.bacc as bacc
import concourse.tile as tile
import concourse.mybir as mybir
from concourse.bass_utils import run_bass_kernel_spmd

# Problem constants (hardcoded per spec).
B = 128
E = 256
M = 64
S = 3
C = 5
SCALES = [4, 8, 16]
EPS = 1e-5

N_CORES = 8
BLOC = B // N_CORES        # 16 samples per core
L2 = BLOC * 2              # 32 rnn2 lanes per free column group (b, e_hi)

FP32 = mybir.dt.float32
BF16 = mybir.dt.bfloat16
AF = mybir.ActivationFunctionType
ALU = mybir.AluOpType

# truncation tolerances (|w|^K bounds; end-to-end error is ~30x smaller
# because tanh' < 1 contracts further)
TOL_STAGE1 = 1e-3
TOL_RNN2 = 3e-3


def _trunc_steps(aw, T, tol):
    if aw < 1e-12:
        return 1
    if aw >= 1.0:
        return T
    return min(T, max(1, int(math.ceil(math.log(tol) / math.log(aw)))))


def _plan(params):
    """Derived scalars: truncation depths, folded coefficients."""
    p = {}
    p["Kt"] = [_trunc_steps(abs(params["whh"][s]), SCALES[s], TOL_STAGE1)
               for s in range(S)]
    p["Km"] = _trunc_steps(abs(params["whh2"]), M, TOL_RNN2)
    # conv folded: u = sum_s c_s h_s + D with c_s = wih2*cw_s,
    # D = wih2*cb + bih2 + bhh2.  Chain through pivot scale pv
    # (largest |c_s|): t2 = sum_s (c_s/c_pv) h_s; rnn2 activation is
    # tanh(c_pv * (h*whh2/c_pv + t2_m) + D).
    c = [params["wih2"] * params["cw"][s] for s in range(S)]
    pv = int(np.argmax([abs(x) for x in c]))
    p["c"] = c
    p["pv"] = pv
    p["D"] = params["wih2"] * params["cb"] + params["bb2"]
    return p


def _build(params, n_devices=N_CORES, km_override=None, kt_override=None):
    nc = bacc.Bacc("TRN2", target_bir_lowering=False, debug=False,
                   enable_asserts=True, num_devices=n_devices)

    plan = _plan(params)
    Km = km_override or plan["Km"]
    Kt = kt_override or plan["Kt"]
    FC = Km * L2               # stage-1 free width: (m, b_loc, e_hi)
    c_s, pv, D = plan["c"], plan["pv"], plan["D"]
    cpv = c_s[pv]
    wih, whh, bb = params["wih"], params["whh"], params["bb"]
    whh2 = params["whh2"]

    # t-block split per scale so the recurrence can start before the
    # whole x tensor lands: first block = first few steps.
    tblocks = []
    for s in range(S):
        t0 = min(Kt[s], 3 if Kt[s] > 4 else Kt[s])
        tblocks.append([t0, Kt[s] - t0] if Kt[s] - t0 > 0 else [Kt[s]])

    a_dram = [
        nc.dram_tensor(f"a{i}", [128 * FC * Kt[i]], BF16, kind="ExternalInput")
        for i in range(S)
    ]
    out_dram = nc.dram_tensor("out", [BLOC, C], FP32, kind="ExternalOutput")

    # One packed constant tensor: [bias(5) | gb(4) | wpack(2C) | fnnb(1) | eye16(16)]
    NCOL = 5 + 4 + 2 * C + 1 + 16
    cpack = np.zeros((128, NCOL), np.float32)
    cpack[:, 0] = bb[0]
    cpack[:, 1] = bb[1]
    cpack[:, 2] = bb[2]
    cpack[:, 3] = D
    cpack[:, 4] = EPS
    cpack[:, 5:7] = params["gamma"].reshape(2, 128).T
    cpack[:, 7:9] = params["beta"].reshape(2, 128).T
    fw = params["fnn_w"]  # (C, E); e = eh*128 + e_lo
    cpack[:, 9:9 + C] = fw[:, :128].T
    cpack[:, 9 + C:9 + 2 * C] = fw[:, 128:].T
    cpack[0:C, 9 + 2 * C] = params["fnn_b"]
    cpack[0:16, 10 + 2 * C:26 + 2 * C] = np.eye(16, dtype=np.float32)
    cpack_c = nc.inline_tensor(cpack, name="cpack")

    from contextlib import ExitStack
    with tile.TileContext(nc) as tc, ExitStack() as ctx:
        singles = ctx.enter_context(tc.tile_pool(name="singles", bufs=1))
        xp = [ctx.enter_context(tc.tile_pool(name=f"x{s}", bufs=1))
              for s in range(S)]
        hp = ctx.enter_context(tc.tile_pool(name="h", bufs=6))
        stp = ctx.enter_context(tc.tile_pool(name="st", bufs=4))
        cvp = ctx.enter_context(tc.tile_pool(name="cv", bufs=2))
        r2p = ctx.enter_context(tc.tile_pool(name="r2", bufs=4))
        smp = ctx.enter_context(tc.tile_pool(name="sm", bufs=2))
        pst = ctx.enter_context(tc.tile_pool(name="pst", bufs=1, space="PSUM"))
        dram = ctx.enter_context(tc.tile_pool(name="dram", bufs=1, space="DRAM"))

        consts = singles.tile([128, NCOL], FP32)
        nc.sync.dma_start(out=consts[:], in_=cpack_c[:])
        bias_c = consts[:, 0:5]
        gb = consts[:, 5:9]
        wpack = consts[:, 9:9 + 2 * C]
        fnnb = consts[0:C, 9 + 2 * C:10 + 2 * C]
        ident = consts[0:16, 10 + 2 * C:26 + 2 * C]

        # x tiles, one per (scale, t-block); DMAs ordered so the first
        # steps of every scale land early.
        xt = [[None] * len(tblocks[s]) for s in range(S)]
        av = [a_dram[s].ap().rearrange("(p ft) -> p ft", p=128)
              for s in range(S)]
        order = [(s, 0) for s in range(S)] + [
            (s, j) for s in range(S) for j in range(1, len(tblocks[s]))]
        for s, j in order:
            tb = tblocks[s]
            t_lo = sum(tb[:j])
            x = xp[s].tile([128, FC, tb[j]], BF16, tag=f"x{s}_{j}",
                           name=f"x{s}_{j}")
            nc.sync.dma_start(
                out=x[:].rearrange("p f t -> p (f t)"),
                in_=av[s][:, FC * t_lo:FC * (t_lo + tb[j])])
            xt[s][j] = x

        def xcol(s, r):
            """x slice for global step r of scale s."""
            tb = tblocks[s]
            j = 0 if r < tb[0] else 1
            return xt[s][j][:, :, r - (0 if j == 0 else tb[0])]

        # ---- stage-1: 3 interleaved recurrences over t ----
        h_cur = [None] * S
        for r in range(max(Kt)):
            for s in range(S):
                if r >= Kt[s]:
                    continue
                hn = hp.tile([128, FC], FP32, tag=f"h{s}", name=f"h{s}")
                if h_cur[s] is None:
                    nc.scalar.activation(hn[:], xcol(s, r), AF.Tanh,
                                         bias=bias_c[:, s:s + 1],
                                         scale=wih[s])
                else:
                    st = stp.tile([128, FC], FP32, tag=f"st{s}",
                                  name=f"st{s}")
                    nc.vector.scalar_tensor_tensor(
                        st[:], h_cur[s][:], whh[s] / wih[s], xcol(s, r),
                        op0=ALU.mult, op1=ALU.add)
                    nc.scalar.activation(hn[:], st[:], AF.Tanh,
                                         bias=bias_c[:, s:s + 1],
                                         scale=wih[s])
                h_cur[s] = hn

        # ---- conv across scales, folded to pivot scale ----
        oth = [s for s in range(S) if s != pv]
        t1 = cvp.tile([128, FC], FP32, tag="t1", name="t1")
        nc.vector.scalar_tensor_tensor(
            t1[:], h_cur[oth[0]][:], c_s[oth[0]] / cpv, h_cur[pv][:],
            op0=ALU.mult, op1=ALU.add)
        t2 = cvp.tile([128, FC], FP32, tag="t2", name="t2")
        nc.vector.scalar_tensor_tensor(
            t2[:], h_cur[oth[1]][:], c_s[oth[1]] / cpv, t1[:],
            op0=ALU.mult, op1=ALU.add)
        u2 = t2[:].rearrange("p (m l) -> p m l", m=Km)

        # ---- rnn2 over m (chain; only last hidden needed) ----
        h2 = None
        feat = smp.tile([128, L2], FP32, tag="feat", name="feat")
        for m in range(Km):
            last = m == Km - 1
            dst = feat[:] if last else r2p.tile(
                [128, L2], FP32, tag="h2", name="h2")[:]
            if h2 is None:
                nc.scalar.activation(dst, u2[:, m, :], AF.Tanh,
                                     bias=bias_c[:, 3:4], scale=cpv)
            else:
                st2 = r2p.tile([128, L2], FP32, tag="st2", name="st2")
                nc.vector.scalar_tensor_tensor(
                    st2[:], h2, whh2 / cpv, u2[:, m, :],
                    op0=ALU.mult, op1=ALU.add)
                nc.scalar.activation(dst, st2[:], AF.Tanh,
                                     bias=bias_c[:, 3:4], scale=cpv)
            h2 = dst

        # ---- BatchNorm stats (partial): sums over local b ----
        featsq = smp.tile([128, L2], FP32, tag="fsq", name="fsq")
        nc.vector.tensor_tensor(featsq[:], feat[:], feat[:], ALU.mult)
        stats = smp.tile([128, 4], FP32, tag="stats", name="stats")
        fv = feat[:].rearrange("p (b eh) -> p eh b", b=BLOC)
        fsv = featsq[:].rearrange("p (b eh) -> p eh b", b=BLOC)
        nc.vector.tensor_reduce(stats[:, 0:2], fv,
                                axis=mybir.AxisListType.X, op=ALU.add)
        nc.vector.tensor_reduce(stats[:, 2:4], fsv,
                                axis=mybir.AxisListType.X, op=ALU.add)

        bin_ = dram.tile([128, 4], FP32, tag="bin")
        bout = dram.tile([128, 4], FP32, tag="bout")
        nc.gpsimd.dma_start(bin_[:], stats[:])
        nc.gpsimd.collective_compute(
            "AllReduce", ALU.add,
            replica_groups=[list(range(N_CORES))],
            ins=[bin_.opt()], outs=[bout.opt()])
        stg = smp.tile([128, 4], FP32, tag="stg")
        nc.gpsimd.dma_start(stg[:], bout[:])

        # ---- BN scale/shift: istd = exp(-0.5*ln(var+eps)) ----
        mean = smp.tile([128, 2], FP32, tag="mean")
        nc.vector.tensor_scalar(mean[:], stg[:, 0:2], 1.0 / B, None, ALU.mult)
        ex2 = smp.tile([128, 2], FP32, tag="ex2")
        nc.vector.tensor_scalar(ex2[:], stg[:, 2:4], 1.0 / B, None, ALU.mult)
        var = smp.tile([128, 2], FP32, tag="var")
        nc.vector.tensor_tensor(var[:], mean[:], mean[:], ALU.mult)
        nc.vector.tensor_tensor(var[:], ex2[:], var[:], ALU.subtract)
        lnv = smp.tile([128, 2], FP32, tag="lnv")
        nc.scalar.activation(lnv[:], var[:], AF.Ln, bias=bias_c[:, 4:5])
        istd = smp.tile([128, 2], FP32, tag="istd")
        nc.scalar.activation(istd[:], lnv[:], AF.Exp, scale=-0.5)
        scl = smp.tile([128, 2], FP32, tag="scl")
        nc.vector.tensor_tensor(scl[:], istd[:], gb[:, 0:2], ALU.mult)
        shf = smp.tile([128, 2], FP32, tag="shf")
        nc.vector.tensor_tensor(shf[:], mean[:], scl[:], ALU.mult)
        nc.vector.tensor_tensor(shf[:], gb[:, 2:4], shf[:], ALU.subtract)

        # ---- normalize + relu fused: relu(scl*f + shf) per e_hi ----
        r = smp.tile([128, L2], FP32, tag="r")
        f3 = feat[:].rearrange("p (b eh) -> p b eh", b=BLOC)
        r3 = r[:].rearrange("p (b eh) -> p b eh", b=BLOC)
        for eh in range(2):
            nc.scalar.activation(r3[:, :, eh], f3[:, :, eh], AF.Relu,
                                 bias=shf[:, eh:eh + 1],
                                 scale=scl[:, eh:eh + 1])

        # ---- FC: logits^T (C, BLOC) = sum_eh Wpack_eh^T @ r[:, :, eh] ----
        tailps = pst.tile([128, 512], FP32, tag="tailps")
        pl = tailps[0:C, 0:BLOC]
        nc.tensor.matmul(pl, wpack[:, 0:C], r3[:, :, 0],
                         start=True, stop=False)
        nc.tensor.matmul(pl, wpack[:, C:2 * C], r3[:, :, 1],
                         start=False, stop=True)
        lt = smp.tile([C, BLOC], FP32, tag="lt")
        nc.vector.tensor_scalar(lt[:], pl, fnnb[:, 0:1], None, ALU.add)

        # ---- transpose to (BLOC, C); softmax along free dim ----
        pt2 = tailps[0:BLOC, 128:128 + C]
        nc.tensor.transpose(pt2, lt[:], ident[0:C, 0:C])
        nmax = smp.tile([BLOC, 1], FP32, tag="nmax")
        nc.vector.tensor_reduce(nmax[:], pt2, axis=mybir.AxisListType.X,
                                op=ALU.max, negate=True)
        esb = smp.tile([BLOC, C], FP32, tag="esb")
        nc.scalar.activation(esb[:], pt2, AF.Exp, bias=nmax[:, 0:1])
        ssum = smp.tile([BLOC, 1], FP32, tag="ssum")
        nc.vector.tensor_reduce(ssum[:], esb[:], axis=mybir.AxisListType.X,
                                op=ALU.add)
        rin = smp.tile([BLOC, 1], FP32, tag="rin")
        nc.vector.reciprocal(rin[:], ssum[:])
        osb = smp.tile([BLOC, C], FP32, tag="osb")
        nc.vector.tensor_scalar(osb[:], esb[:], rin[:, 0:1], None, ALU.mult)
        nc.sync.dma_start(out=out_dram[:], in_=osb[:])

    nc.compile()
    return nc, Km, Kt, [list(tb) for tb in
                        ([tblocks[s] for s in range(S)])]


def _gather_core(a_list, k, Km, Kt, tblocks):
    """Host-side gather for core k: bf16, layout [e_lo, tblock, (m b eh), t]."""
    out = []
    for s in range(S):
        T = SCALES[s]
        A = np.asarray(a_list[s])[:, :, 0].reshape(B, M, 2, 128, T)
        Sv = A[k * BLOC:(k + 1) * BLOC, M - Km:, :, :, T - Kt[s]:]
        # [b, m, eh, e_lo, t] -> [e_lo, m, b, eh, t]
        Sv = np.transpose(Sv, (3, 1, 0, 2, 4))
        parts = []
        t_lo = 0
        for tb in tblocks[s]:
            blk = Sv[..., t_lo:t_lo + tb]
            parts.append(np.ascontiguousarray(blk).reshape(128, -1))
            t_lo += tb
        full = np.concatenate(parts, axis=1)
        out.append(full.astype(ml_dtypes.bfloat16).reshape(-1))
    return out


def kernel(a0, a1, a2, rnn1_wih, rnn1_whh, rnn1_bih, rnn1_bhh,
           conv_w, conv_b, rnn2_wih, rnn2_whh, rnn2_bih, rnn2_bhh,
           norm_gamma, norm_beta, fnn_w, fnn_b, _bench=None,
           _km=None, _kt=None):
    params = {
        "wih": [float(rnn1_wih[s]) for s in range(S)],
        "whh": [float(rnn1_whh[s]) for s in range(S)],
        "bb": [float(rnn1_bih[s]) + float(rnn1_bhh[s]) for s in range(S)],
        "cw": [float(conv_w[s]) for s in range(S)],
        "cb": float(conv_b[0]),
        "wih2": float(rnn2_wih[0]),
        "whh2": float(rnn2_whh[0]),
        "bb2": float(rnn2_bih[0]) + float(rnn2_bhh[0]),
        "gamma": np.asarray(norm_gamma, np.float32),
        "beta": np.asarray(norm_beta, np.float32),
        "fnn_w": np.asarray(fnn_w, np.float32),
        "fnn_b": np.asarray(fnn_b, np.float32),
    }
    nc, Km, Kt, tblocks = _build(params, km_override=_km, kt_override=_kt)

    in_maps = []
    for k in range(N_CORES):
        arrs = _gather_core((a0, a1, a2), k, Km, Kt, tblocks)
        in_maps.append({f"a{i}": arrs[i] for i in range(S)})

    kw = dict(_bench) if _bench else {}
    res = run_bass_kernel_spmd(nc, in_maps, core_ids=list(range(N_CORES)),
                               **kw)
    out = np.concatenate([res.results[k]["out"] for k in range(N_CORES)],
                         axis=0)
    if _bench is not None:
        kernel.last_result = res
    return out


# revision 13
# speedup vs baseline: 2.4023x; 1.0245x over previous
"""Trainium2 Bass kernel for nn_CLS_1889785610440.

Pipeline (per reference.py):
  3 scalar Elman RNNs over T in {4,8,16} for N=B*M*E lanes -> last hidden
  -> 1x3 conv over scales -> scalar RNN over M=64 -> BatchNorm1d (batch
  stats) -> ReLU -> Linear(E,C) -> softmax.

Key optimizations vs the v1 baseline:
  * Truncation-aware LOADING: the rnn2 recurrence over m contracts by
    |whh2| (=0.61) per step, so only the last Km ~ 12 of 64 m-positions
    influence the output above ~3e-3; stage-1 recurrences likewise only
    need their last Kt_s steps (|whh_s|^Kt_s <= 1e-3).  Lanes with
    m < M-Km and time-steps t < T-Kt_s are never uploaded or computed.
    (measured end-to-end error 2.2e-3 vs the 2e-2 gate)
  * bf16 upload: inputs are quantized host-side to bf16 (recurrence
    arithmetic stays fp32 on-chip), halving DMA bytes.
  * Host-side gather picks the lane order so stage-1 partitions are
    e_lo directly: free dim = (m, b_loc, e_hi).  The conv output IS the
    rnn2 input buffer - no PE transposes / scatters at all.
  * Stage-1 step on DVE+ACT: st = h*(whh/wih) + x_t (one
    scalar_tensor_tensor), h = tanh(wih*st + b) (one activation with
    folded scale+bias).  No matmuls, PSUM untouched until the FC.
  * Conv + rnn2 input affine folded into 2 DVE ops (pivot scale) and
    the rnn2 activation scale/bias.
  * BN normalize+relu fused into one ACT op per e_hi via per-partition
    scale/bias (relu(scl*f + shf)).
  * Single ACT table switch (Tanh -> Ln/Exp set) for the whole kernel.

Sharding: data-parallel over B=128 -> 16 samples/core.  Only the
BatchNorm statistics cross cores (one 2KB AllReduce).
"""

import math

import numpy as np
import ml_dtypes

import concourse.bacc as bacc
import concourse.tile as tile
import concourse.mybir as mybir
from concourse.bass_utils import run_bass_kernel_spmd

# Problem constants (hardcoded per spec).
B = 128
E = 256
M = 64
S = 3
C = 5
SCALES = [4, 8, 16]
EPS = 1e-5

N_CORES = 8
BLOC = B // N_CORES        # 16 samples per core
L2 = BLOC * 2              # 32 rnn2 lanes per free column group (b, e_hi)

FP32 = mybir.dt.float32
BF16 = mybir.dt.bfloat16
AF = mybir.ActivationFunctionType
ALU = mybir.AluOpType

# truncation tolerances (|w|^K bounds; end-to-end error is ~30x smaller
# because tanh' < 1 contracts further)
TOL_STAGE1 = 1e-3
TOL_RNN2 = 3e-3


def _trunc_steps(aw, T, tol):
    if aw < 1e-12:
        return 1
    if aw >= 1.0:
        return T
    return min(T, max(1, int(math.ceil(math.log(tol) / math.log(aw)))))


def _plan(params):
    """Derived scalars: truncation depths, folded coefficients."""
    p = {}
    p["Kt"] = [_trunc_steps(abs(params["whh"][s]), SCALES[s], TOL_STAGE1)
               for s in range(S)]
    p["Km"] = _trunc_steps(abs(params["whh2"]), M, TOL_RNN2)
    # conv folded: u = sum_s c_s h_s + D with c_s = wih2*cw_s,
    # D = wih2*cb + bih2 + bhh2.  Chain through pivot scale pv
    # (largest |c_s|): t2 = sum_s (c_s/c_pv) h_s; rnn2 activation is
    # tanh(c_pv * (h*whh2/c_pv + t2_m) + D).
    c = [params["wih2"] * params["cw"][s] for s in range(S)]
    pv = int(np.argmax([abs(x) for x in c]))
    p["c"] = c
    p["pv"] = pv
    p["D"] = params["wih2"] * params["cb"] + params["bb2"]
    return p


def _build(params, n_devices=N_CORES, km_override=None, kt_override=None):
    nc = bacc.Bacc("TRN2", target_bir_lowering=False, debug=False,
                   enable_asserts=True, num_devices=n_devices)

    plan = _plan(params)
    Km = km_override or plan["Km"]
    Kt = kt_override or plan["Kt"]
    FC = Km * L2               # stage-1 free width: (m, b_loc, e_hi)
    c_s, pv, D = plan["c"], plan["pv"], plan["D"]
    cpv = c_s[pv]
    wih, whh, bb = params["wih"], params["whh"], params["bb"]
    whh2 = params["whh2"]

    # t-block split per scale so the recurrence can start before the
    # whole x tensor lands: blocks of [1, 2, rest] steps.
    tblocks = []
    for s in range(S):
        blks, rem = [], Kt[s]
        for want in (1, 2):
            if rem > want:
                blks.append(want)
                rem -= want
        blks.append(rem)
        tblocks.append(blks)

    a_dram = [
        nc.dram_tensor(f"a{i}", [128 * FC * Kt[i]], BF16, kind="ExternalInput")
        for i in range(S)
    ]
    out_dram = nc.dram_tensor("out", [BLOC, C], FP32, kind="ExternalOutput")

    # One packed constant tensor: [bias(5) | gb(4) | wpack(2C) | fnnb(1) | eye16(16)]
    NCOL = 5 + 4 + 2 * C + 1 + 16
    cpack = np.zeros((128, NCOL), np.float32)
    cpack[:, 0] = bb[0]
    cpack[:, 1] = bb[1]
    cpack[:, 2] = bb[2]
    cpack[:, 3] = D
    cpack[:, 4] = EPS
    cpack[:, 5:7] = params["gamma"].reshape(2, 128).T
    cpack[:, 7:9] = params["beta"].reshape(2, 128).T
    fw = params["fnn_w"]  # (C, E); e = eh*128 + e_lo
    cpack[:, 9:9 + C] = fw[:, :128].T
    cpack[:, 9 + C:9 + 2 * C] = fw[:, 128:].T
    cpack[0:C, 9 + 2 * C] = params["fnn_b"]
    cpack[0:16, 10 + 2 * C:26 + 2 * C] = np.eye(16, dtype=np.float32)
    cpack_c = nc.inline_tensor(cpack, name="cpack")

    from contextlib import ExitStack
    with tile.TileContext(nc) as tc, ExitStack() as ctx:
        singles = ctx.enter_context(tc.tile_pool(name="singles", bufs=1))
        xp = [ctx.enter_context(tc.tile_pool(name=f"x{s}", bufs=1))
              for s in range(S)]
        hp = ctx.enter_context(tc.tile_pool(name="h", bufs=6))
        stp = ctx.enter_context(tc.tile_pool(name="st", bufs=4))
        cvp = ctx.enter_context(tc.tile_pool(name="cv", bufs=2))
        r2p = ctx.enter_context(tc.tile_pool(name="r2", bufs=4))
        smp = ctx.enter_context(tc.tile_pool(name="sm", bufs=2))
        pst = ctx.enter_context(tc.tile_pool(name="pst", bufs=1, space="PSUM"))
        dram = ctx.enter_context(tc.tile_pool(name="dram", bufs=1, space="DRAM"))

        consts = singles.tile([128, NCOL], FP32)
        nc.scalar.dma_start(out=consts[:], in_=cpack_c[:])
        bias_c = consts[:, 0:5]
        gb = consts[:, 5:9]
        wpack = consts[:, 9:9 + 2 * C]
        fnnb = consts[0:C, 9 + 2 * C:10 + 2 * C]
        ident = consts[0:16, 10 + 2 * C:26 + 2 * C]

        # x tiles, one per (scale, t-block), t-major so each step's
        # slice is contiguous.  DMAs are spread over several engine
        # rings so their issue doesn't serialize, ordered so the first
        # steps of every scale land early.
        xt = [[None] * len(tblocks[s]) for s in range(S)]
        av = [a_dram[s].ap().rearrange("(p ft) -> p ft", p=128)
              for s in range(S)]
        order = ([(s, 0) for s in range(S)]
                 + [(s, 1) for s in range(S) if len(tblocks[s]) > 1]
                 + [(s, 2) for s in range(S) if len(tblocks[s]) > 2])
        rings = [nc.sync, nc.scalar, nc.gpsimd]
        for i, (s, j) in enumerate(order):
            tb = tblocks[s]
            t_lo = sum(tb[:j])
            x = xp[s].tile([128, tb[j], FC], BF16, tag=f"x{s}_{j}",
                           name=f"x{s}_{j}")
            rings[i % len(rings)].dma_start(
                out=x[:].rearrange("p t f -> p (t f)"),
                in_=av[s][:, FC * t_lo:FC * (t_lo + tb[j])])
            xt[s][j] = x

        def xcol(s, r):
            """x slice for global step r of scale s (contiguous)."""
            lo = 0
            for j, tb in enumerate(tblocks[s]):
                if r < lo + tb:
                    return xt[s][j][:, r - lo, :]
                lo += tb
            raise AssertionError

        # ---- stage-1: 3 interleaved recurrences over t, with the
        # conv partial fused in as soon as two scales complete ----
        # conv folded to pivot: t2 = sum_s (c_s/c_pv) h_s; the scale
        # finishing LAST is chained in last.
        fin_round = sorted(range(S), key=lambda s: (Kt[s], s))
        early2, last1 = fin_round[:2], fin_round[2]
        cmax = max(abs(x) for x in c_s)
        if (abs(c_s[early2[1]]) < 1e-6 * cmax
                or abs(c_s[last1]) < 1e-6 * cmax):
            # degenerate conv weights: chain in increasing-|c| order so
            # every ratio is <= ~1 (loses the early-combine overlap)
            by_mag = sorted(range(S), key=lambda s: abs(c_s[s]))
            early2, last1 = by_mag[:2], by_mag[2]
        h_cur = [None] * S
        t1 = cvp.tile([128, FC], FP32, tag="t1", name="t1")
        t2 = cvp.tile([128, FC], FP32, tag="t2", name="t2")
        for r in range(max(Kt)):
            for s in range(S):
                if r >= Kt[s]:
                    continue
                hn = hp.tile([128, FC], FP32, tag=f"h{s}", name=f"h{s}")
                if h_cur[s] is None:
                    nc.scalar.activation(hn[:], xcol(s, r), AF.Tanh,
                                         bias=bias_c[:, s:s + 1],
                                         scale=wih[s])
                else:
                    st = stp.tile([128, FC], FP32, tag=f"st{s}",
                                  name=f"st{s}")
                    nc.vector.scalar_tensor_tensor(
                        st[:], h_cur[s][:], whh[s] / wih[s], xcol(s, r),
                        op0=ALU.mult, op1=ALU.add)
                    nc.scalar.activation(hn[:], st[:], AF.Tanh,
                                         bias=bias_c[:, s:s + 1],
                                         scale=wih[s])
                h_cur[s] = hn
            if r == max(Kt[early2[0]], Kt[early2[1]]) - 1:
                # both early scales done: combine them now (overlaps
                # the remaining lone-scale rounds)
                a, b = early2
                nc.vector.scalar_tensor_tensor(
                    t1[:], h_cur[a][:], c_s[a] / c_s[b], h_cur[b][:],
                    op0=ALU.mult, op1=ALU.add)
        nc.vector.scalar_tensor_tensor(
            t2[:], t1[:], c_s[early2[1]] / c_s[last1], h_cur[last1][:],
            op0=ALU.mult, op1=ALU.add)
        # t2 = sum_s (c_s/c_last) h_s; rnn2 activation scale is c_last
        cpv = c_s[last1]
        u2 = t2[:].rearrange("p (m l) -> p m l", m=Km)

        # ---- rnn2 over m (chain; only last hidden needed) ----
        h2 = None
        feat = smp.tile([128, L2], FP32, tag="feat", name="feat")
        for m in range(Km):
            last = m == Km - 1
            dst = feat[:] if last else r2p.tile(
                [128, L2], FP32, tag="h2", name="h2")[:]
            if h2 is None:
                nc.scalar.activation(dst, u2[:, m, :], AF.Tanh,
                                     bias=bias_c[:, 3:4], scale=cpv)
            else:
                st2 = r2p.tile([128, L2], FP32, tag="st2", name="st2")
                nc.vector.scalar_tensor_tensor(
                    st2[:], h2, whh2 / cpv, u2[:, m, :],
                    op0=ALU.mult, op1=ALU.add)
                nc.scalar.activation(dst, st2[:], AF.Tanh,
                                     bias=bias_c[:, 3:4], scale=cpv)
            h2 = dst

        # ---- BatchNorm stats (partial): sums over local b ----
        featsq = smp.tile([128, L2], FP32, tag="fsq", name="fsq")
        nc.vector.tensor_tensor(featsq[:], feat[:], feat[:], ALU.mult)
        stats = smp.tile([128, 4], FP32, tag="stats", name="stats")
        fv = feat[:].rearrange("p (b eh) -> p eh b", b=BLOC)
        fsv = featsq[:].rearrange("p (b eh) -> p eh b", b=BLOC)
        nc.vector.tensor_reduce(stats[:, 0:2], fv,
                                axis=mybir.AxisListType.X, op=ALU.add)
        nc.vector.tensor_reduce(stats[:, 2:4], fsv,
                                axis=mybir.AxisListType.X, op=ALU.add)

        bin_ = dram.tile([128, 4], FP32, tag="bin")
        bout = dram.tile([128, 4], FP32, tag="bout")
        nc.sync.dma_start(bin_[:], stats[:])
        nc.gpsimd.collective_compute(
            "AllReduce", ALU.add,
            replica_groups=[list(range(N_CORES))],
            ins=[bin_.opt()], outs=[bout.opt()])
        stg = smp.tile([128, 4], FP32, tag="stg")
        nc.sync.dma_start(stg[:], bout[:])

        # ---- BN scale/shift: istd = exp(-0.5 ln(var+eps)) ----
        mex = smp.tile([128, 4], FP32, tag="mex")
        # cols 0:2 = mean; cols 2:4 = E[x^2] + eps (eps folded here)
        nc.vector.tensor_scalar(mex[:, 0:2], stg[:, 0:2], 1.0 / B, None,
                                ALU.mult)
        nc.vector.tensor_scalar(mex[:, 2:4], stg[:, 2:4], 1.0 / B, EPS,
                                ALU.mult, op1=ALU.add)
        mean = mex[:, 0:2]
        var = smp.tile([128, 2], FP32, tag="var")
        nc.vector.tensor_tensor(var[:], mean, mean, ALU.mult)
        nc.vector.tensor_tensor(var[:], mex[:, 2:4], var[:], ALU.subtract)
        lnv = smp.tile([128, 2], FP32, tag="lnv")
        nc.scalar.activation(lnv[:], var[:], AF.Ln)
        istd = smp.tile([128, 2], FP32, tag="istd")
        nc.scalar.activation(istd[:], lnv[:], AF.Exp, scale=-0.5)
        scl = smp.tile([128, 2], FP32, tag="scl")
        nc.vector.tensor_tensor(scl[:], istd[:], gb[:, 0:2], ALU.mult)
        shf = smp.tile([128, 2], FP32, tag="shf")
        nc.vector.tensor_tensor(shf[:], mean, scl[:], ALU.mult)
        nc.vector.tensor_tensor(shf[:], gb[:, 2:4], shf[:], ALU.subtract)

        # ---- normalize + relu fused: relu(scl*f + shf) per e_hi ----
        r = smp.tile([128, L2], FP32, tag="r")
        f3 = feat[:].rearrange("p (b eh) -> p b eh", b=BLOC)
        r3 = r[:].rearrange("p (b eh) -> p b eh", b=BLOC)
        for eh in range(2):
            nc.scalar.activation(r3[:, :, eh], f3[:, :, eh], AF.Relu,
                                 bias=shf[:, eh:eh + 1],
                                 scale=scl[:, eh:eh + 1])

        # ---- FC: logits^T (C, BLOC) = sum_eh Wpack_eh^T @ r[:, :, eh] ----
        tailps = pst.tile([128, 512], FP32, tag="tailps")
        pl = tailps[0:C, 0:BLOC]
        nc.tensor.matmul(pl, wpack[:, 0:C], r3[:, :, 0],
                         start=True, stop=False)
        nc.tensor.matmul(pl, wpack[:, C:2 * C], r3[:, :, 1],
                         start=False, stop=True)
        lt = smp.tile([C, BLOC], FP32, tag="lt")
        nc.vector.tensor_scalar(lt[:], pl, fnnb[:, 0:1], None, ALU.add)

        # ---- transpose to (BLOC, C); softmax along free dim ----
        pt2 = tailps[0:BLOC, 128:128 + C]
        nc.tensor.transpose(pt2, lt[:], ident[0:C, 0:C])
        nmax = smp.tile([BLOC, 1], FP32, tag="nmax")
        nc.vector.tensor_reduce(nmax[:], pt2, axis=mybir.AxisListType.X,
                                op=ALU.max, negate=True)
        esb = smp.tile([BLOC, C], FP32, tag="esb")
        nc.scalar.activation(esb[:], pt2, AF.Exp, bias=nmax[:, 0:1])
        ssum = smp.tile([BLOC, 1], FP32, tag="ssum")
        nc.vector.tensor_reduce(ssum[:], esb[:], axis=mybir.AxisListType.X,
                                op=ALU.add)
        rin = smp.tile([BLOC, 1], FP32, tag="rin")
        nc.vector.reciprocal(rin[:], ssum[:])
        osb = smp.tile([BLOC, C], FP32, tag="osb")
        nc.vector.tensor_scalar(osb[:], esb[:], rin[:, 0:1], None, ALU.mult)
        nc.sync.dma_start(out=out_dram[:], in_=osb[:])

    nc.compile()
    return nc, Km, Kt, [list(tb) for tb in
                        ([tblocks[s] for s in range(S)])]


def _gather_core(a_list, k, Km, Kt, tblocks):
    """Host-side gather for core k: bf16, layout [e_lo, tblock, t, (m b eh)]."""
    out = []
    for s in range(S):
        T = SCALES[s]
        A = np.asarray(a_list[s])[:, :, 0].reshape(B, M, 2, 128, T)
        Sv = A[k * BLOC:(k + 1) * BLOC, M - Km:, :, :, T - Kt[s]:]
        # [b, m, eh, e_lo, t] -> [e_lo, t, m, b, eh]
        Sv = np.transpose(Sv, (3, 4, 1, 0, 2))
        parts = []
        t_lo = 0
        for tb in tblocks[s]:
            blk = Sv[:, t_lo:t_lo + tb]
            parts.append(np.ascontiguousarray(blk).reshape(128, -1))
            t_lo += tb
        full = np.concatenate(parts, axis=1)
        out.append(full.astype(ml_dtypes.bfloat16).reshape(-1))
    return out


def kernel(a0, a1, a2, rnn1_wih, rnn1_whh, rnn1_bih, rnn1_bhh,
           conv_w, conv_b, rnn2_wih, rnn2_whh, rnn2_bih, rnn2_bhh,
           norm_gamma, norm_beta, fnn_w, fnn_b, _bench=None,
           _km=None, _kt=None):
    params = {
        "wih": [float(rnn1_wih[s]) for s in range(S)],
        "whh": [float(rnn1_whh[s]) for s in range(S)],
        "bb": [float(rnn1_bih[s]) + float(rnn1_bhh[s]) for s in range(S)],
        "cw": [float(conv_w[s]) for s in range(S)],
        "cb": float(conv_b[0]),
        "wih2": float(rnn2_wih[0]),
        "whh2": float(rnn2_whh[0]),
        "bb2": float(rnn2_bih[0]) + float(rnn2_bhh[0]),
        "gamma": np.asarray(norm_gamma, np.float32),
        "beta": np.asarray(norm_beta, np.float32),
        "fnn_w": np.asarray(fnn_w, np.float32),
        "fnn_b": np.asarray(fnn_b, np.float32),
    }
    nc, Km, Kt, tblocks = _build(params, km_override=_km, kt_override=_kt)

    in_maps = []
    for k in range(N_CORES):
        arrs = _gather_core((a0, a1, a2), k, Km, Kt, tblocks)
        in_maps.append({f"a{i}": arrs[i] for i in range(S)})

    kw = dict(_bench) if _bench else {}
    res = run_bass_kernel_spmd(nc, in_maps, core_ids=list(range(N_CORES)),
                               **kw)
    out = np.concatenate([res.results[k]["out"] for k in range(N_CORES)],
                         axis=0)
    if _bench is not None:
        kernel.last_result = res
    return out
